# revision 7
# baseline (speedup 1.0000x reference)
"""Trainium2 Bass kernel for nn_HeatmapLayer: separable Gaussian heatmaps.

Reference math (per batch b, class c):
    mx = labels[b, 2c] * H ; my = labels[b, 2c+1] * W          (H = W = 384)
    sigma = H * exp(log_weight)
    dx2[h] = (h - mx)^2 / sigma        ; normalized by its min over h
    dy2[w] = (w - my)^2 / (20 * sigma) ; normalized by its min over w
    out[b,c,h,w] = exp(-0.5*(dx2[h] + dy2[w])) = ex[h] * ey[w]

So each (b,c) heatmap is a rank-1 outer product of two 384-length
profiles.  Strategy per core (pure data parallel over batch, 2 batches =
12 (b,c) pairs per core):
  1. tiny setup on [12, 384] tiles: iota grid, squared distances,
     min-reduction, exp -> EX, EY profiles (partition = pair index)
  2. 36 rank-1 PE matmuls  ex_chunk[1,128]^T @ ey[1,384] -> PSUM [128,384]
  3. PSUM->SBUF copies alternating Vector/Scalar engines
  4. one ~576KB HWDGE DMA per pair SBUF -> DRAM
x is only used for its shape; it is never transferred to the device.
"""

import numpy as np
from contextlib import ExitStack

import concourse.bacc as bacc
import concourse.bass as bass
import concourse.tile as tile
from concourse import mybir
from concourse.bass_utils import run_bass_kernel_spmd

B, CH, H, W = 16, 3, 384, 384
NCLS = 6
N_CORES = 8
BPC = B // N_CORES            # batches per core = 2
PAIRS = BPC * NCLS            # (b,c) pairs per core = 12
P = 128
CHUNKS = H // P               # 3
LN_H = float(np.log(H))
F32 = mybir.dt.float32


def build_bass() -> bass.Bass:
    nc = bacc.Bacc("TRN2", target_bir_lowering=False, debug=False,
                   num_devices=N_CORES)
    labels = nc.dram_tensor("labels", [BPC, 2 * NCLS], F32,
                            kind="ExternalInput")
    logw = nc.dram_tensor("log_weight", [1, 1], F32, kind="ExternalInput")
    out = nc.dram_tensor("out", [PAIRS * H, W], F32, kind="ExternalOutput")

    with ExitStack() as ctx:
        tc = ctx.enter_context(tile.TileContext(nc))
        singles = ctx.enter_context(tc.tile_pool(name="singles", bufs=1))
        psum = ctx.enter_context(tc.tile_pool(name="psum", bufs=8,
                                              space="PSUM"))
        stage = ctx.enter_context(tc.tile_pool(name="stage", bufs=6))

        # ---- load inputs -------------------------------------------------
        # labels [2,12] -> [12,2] tile: partition p=(b*6+c), free=(mx,my).
        # flat offset of (b, 2c+t) is 2*(b*6+c)+t = 2p+t.
        lab = singles.tile([PAIRS, 2], F32)
        nc.sync.dma_start(
            out=lab,
            in_=labels[:, :].rearrange("b (q two) -> (b q) two", two=2),
        )
        # log_weight scalar broadcast to all 12 pair-partitions.
        lwb = singles.tile([PAIRS, 1], F32)
        nc.gpsimd.dma_start(out=lwb, in_=logw[:, :].to_broadcast((PAIRS, 1)))

        # ---- grid and per-pair scalars ----------------------------------
        iot_i = singles.tile([PAIRS, W], mybir.dt.int32)
        nc.gpsimd.iota(iot_i, pattern=[[1, W]], base=0, channel_multiplier=0)
        iot = singles.tile([PAIRS, W], F32)
        nc.vector.tensor_copy(out=iot, in_=iot_i)

        # neg_m[:,0] = -mx, neg_m[:,1] = -my
        neg_m = singles.tile([PAIRS, 2], F32)
        nc.vector.tensor_scalar_mul(out=neg_m, in0=lab, scalar1=-float(H))

        # inv_s = 1/sigma = exp(-log_weight - ln(H))
        nlw = singles.tile([PAIRS, 1], F32)
        nc.vector.tensor_scalar(out=nlw, in0=lwb, scalar1=-1.0,
                                scalar2=-LN_H, op0=mybir.AluOpType.mult,
                                op1=mybir.AluOpType.add)
        inv_s = singles.tile([PAIRS, 1], F32)
        nc.scalar.activation(out=inv_s, in_=nlw,
                             func=mybir.ActivationFunctionType.Exp,
                             bias=0.0, scale=1.0)
        # sc columns: 0: -inv_s/2, 1: -inv_s/40, 2: +inv_s/2, 3: +inv_s/40
        sc = singles.tile([PAIRS, 4], F32)
        for i, m in enumerate((-0.5, -0.025, 0.5, 0.025)):
            nc.vector.tensor_scalar_mul(out=sc[:, i:i + 1], in0=inv_s,
                                        scalar1=m)

        # ---- 1D profiles EX, EY -----------------------------------------
        sqx = singles.tile([PAIRS, W], F32)
        sqy = singles.tile([PAIRS, W], F32)
        nc.scalar.activation(out=sqx, in_=iot,
                             func=mybir.ActivationFunctionType.Square,
                             bias=neg_m[:, 0:1], scale=1.0)
        nc.scalar.activation(out=sqy, in_=iot,
                             func=mybir.ActivationFunctionType.Square,
                             bias=neg_m[:, 1:2], scale=1.0)
        mnx = singles.tile([PAIRS, 1], F32)
        mny = singles.tile([PAIRS, 1], F32)
        nc.vector.tensor_reduce(out=mnx, in_=sqx, axis=mybir.AxisListType.X,
                                op=mybir.AluOpType.min)
        nc.vector.tensor_reduce(out=mny, in_=sqy, axis=mybir.AxisListType.X,
                                op=mybir.AluOpType.min)
        bx = singles.tile([PAIRS, 1], F32)
        by = singles.tile([PAIRS, 1], F32)
        nc.vector.tensor_mul(out=bx, in0=mnx, in1=sc[:, 2:3])
        nc.vector.tensor_mul(out=by, in0=mny, in1=sc[:, 3:4])

        ex = singles.tile([PAIRS, W], F32)
        ey = singles.tile([PAIRS, W], F32)
        nc.scalar.activation(out=ex, in_=sqx,
                             func=mybir.ActivationFunctionType.Exp,
                             bias=bx, scale=sc[:, 0:1])
        nc.scalar.activation(out=ey, in_=sqy,
                             func=mybir.ActivationFunctionType.Exp,
                             bias=by, scale=sc[:, 1:2])

        # PE weights must start at partition 0/32/64 -> repack the profiles
        # onto partition 0 with pairs along the free dim.
        exr = singles.tile([1, PAIRS, W], F32)
        eyr = singles.tile([1, PAIRS, W], F32)
        nc.sync.dma_start(out=exr, in_=ex)
        nc.sync.dma_start(out=eyr, in_=ey)

        # ---- outer products + writeback ---------------------------------
        i = 0
        for p in range(PAIRS):
            st = stage.tile([P, CHUNKS, W], F32)
            for c in range(CHUNKS):
                ps = psum.tile([P, W], F32)
                nc.tensor.matmul(ps, exr[0:1, p, c * P:(c + 1) * P],
                                 eyr[0:1, p, :], start=True, stop=True)
                if i % 2 == 0:
                    nc.vector.tensor_copy(out=st[:, c, :], in_=ps)
                else:
                    nc.scalar.copy(out=st[:, c, :], in_=ps)
                i += 1
            # rows of pair p are h = c*128 + par ; DRAM side iterates
            # (par, c, w) to match the SBUF tile layout.
            nc.sync.dma_start(
                out=out[p * H:(p + 1) * H, :].rearrange(
                    "(c par) w -> par c w", par=P),
                in_=st,
            )
    nc.finalize()
    return nc


LAST_RESULTS = None  # BassKernelResults of the most recent kernel() call


def kernel(x: np.ndarray, labels: np.ndarray,
           log_weight: np.ndarray, **run_kwargs) -> np.ndarray:
    global LAST_RESULTS
    del x  # only its (hardcoded) shape matters
    nc = build_bass()
    labels = np.ascontiguousarray(labels, dtype=np.float32)
    lw = np.ascontiguousarray(log_weight, dtype=np.float32).reshape(1, 1)
    in_maps = [
        {"labels": labels[i * BPC:(i + 1) * BPC], "log_weight": lw}
        for i in range(N_CORES)
    ]
    res = run_bass_kernel_spmd(nc, in_maps, core_ids=list(range(N_CORES)),
                               **run_kwargs)
    LAST_RESULTS = res
    outs = [r["out"].reshape(BPC, NCLS, H, W) for r in res.results]
    return np.concatenate(outs, axis=0)


if __name__ == "__main__":
    rng = np.random.default_rng(0)
    x = rng.standard_normal((B, CH, H, W), dtype=np.float32)
    labels = rng.random((B, 2 * NCLS), dtype=np.float32)
    lw = rng.random((1, 1, 1, 1), dtype=np.float32)
    y = kernel(x=x, labels=labels, log_weight=lw)
    print(y.shape, y.dtype, y.min(), y.max())


# revision 9
# speedup vs baseline: 1.6626x; 1.6626x over previous
"""Trainium2 Bass kernel for nn_HeatmapLayer: separable Gaussian heatmaps.

Reference math (per batch b, class c):
    mx = labels[b, 2c] * H ; my = labels[b, 2c+1] * W          (H = W = 384)
    sigma = H * exp(log_weight)
    dx2[h] = (h - mx)^2 / sigma        ; normalized by its min over h
    dy2[w] = (w - my)^2 / (20 * sigma) ; normalized by its min over w
    out[b,c,h,w] = exp(-0.5*(dx2[h] + dy2[w])) = ex[h] * ey[w]

Each (b,c) heatmap is a rank-1 outer product of two 384-length
profiles.  Per core (pure data parallel over batch: 2 batches = 12
(b,c) pairs per core):

  * x-profiles (with both min-normalization corrections folded in) are
    computed on a [12, 384] tile (partition = pair) and PE-transposed
    into per-partition scalars EXT[128, 3, 12].
  * y-profiles are recomputed redundantly on all 128 partitions
    (iota grid + Square + Exp on the Scalar engine, with per-pair
    scalars broadcast via 0-stride DMA of the tiny inputs), giving
    EYB[128, 384] per pair.
  * outer product = 36 Vector-engine tensor_scalar multiplies
    (fp32 SBUF 2x mode), one [128, 384] chunk each.
  * one ~576KB HWDGE DMA per pair, SBUF -> DRAM (the ~20us roofline).

x is only used for its shape; it is never transferred to the device.
"""

import numpy as np
from contextlib import ExitStack

import concourse.bacc as bacc
import concourse.bass as bass
import concourse.tile as tile
from concourse import mybir
from concourse.bass_utils import run_bass_kernel_spmd
from concourse.masks import make_identity

B, CH, H, W = 16, 3, 384, 384
NCLS = 6
N_CORES = 8
BPC = B // N_CORES            # batches per core = 2
PAIRS = BPC * NCLS            # (b,c) pairs per core = 12
P = 128
CHUNKS = H // P               # 3
LN_H = float(np.log(H))
F32 = mybir.dt.float32


def build_bass() -> bass.Bass:
    nc = bacc.Bacc("TRN2", target_bir_lowering=False, debug=False,
                   num_devices=N_CORES)
    labels = nc.dram_tensor("labels", [BPC, 2 * NCLS], F32,
                            kind="ExternalInput")
    logw = nc.dram_tensor("log_weight", [1, 1], F32, kind="ExternalInput")
    out = nc.dram_tensor("out", [PAIRS * H, W], F32, kind="ExternalOutput")

    with ExitStack() as ctx:
        tc = ctx.enter_context(tile.TileContext(nc))
        singles = ctx.enter_context(tc.tile_pool(name="singles", bufs=1))
        psum = ctx.enter_context(tc.tile_pool(name="psum", bufs=3,
                                              space="PSUM"))
        ybuf = ctx.enter_context(tc.tile_pool(name="ybuf", bufs=3))
        stage = ctx.enter_context(tc.tile_pool(name="stage", bufs=6))

        # ---- small-tile setup: pairs on partitions 0..11 -----------------
        lab = singles.tile([PAIRS, 2], F32)
        nc.sync.dma_start(
            out=lab,
            in_=labels[:, :].rearrange("b (q two) -> (b q) two", two=2),
        )
        lwb = singles.tile([PAIRS, 1], F32)
        nc.gpsimd.dma_start(out=lwb, in_=logw[:, :].to_broadcast((PAIRS, 1)))

        iot_i = singles.tile([PAIRS, W], mybir.dt.int32)
        nc.gpsimd.iota(iot_i, pattern=[[1, W]], base=0, channel_multiplier=0)
        iot = singles.tile([PAIRS, W], F32)
        nc.vector.tensor_copy(out=iot, in_=iot_i)

        # neg_m[:,0] = -mx, neg_m[:,1] = -my
        neg_m = singles.tile([PAIRS, 2], F32)
        nc.vector.tensor_scalar_mul(out=neg_m, in0=lab, scalar1=-float(H))

        # inv_s = 1/sigma = exp(-log_weight - ln(H))
        nlw = singles.tile([PAIRS, 1], F32)
        nc.vector.tensor_scalar(out=nlw, in0=lwb, scalar1=-1.0,
                                scalar2=-LN_H, op0=mybir.AluOpType.mult,
                                op1=mybir.AluOpType.add)
        inv_s = singles.tile([PAIRS, 1], F32)
        nc.scalar.activation(out=inv_s, in_=nlw,
                             func=mybir.ActivationFunctionType.Exp,
                             bias=0.0, scale=1.0)
        # sc columns: 0: -inv_s/2 (x exp scale), 1: +inv_s/2, 2: +inv_s/40
        sc = singles.tile([PAIRS, 3], F32)
        for i, m in enumerate((-0.5, 0.5, 0.025)):
            nc.vector.tensor_scalar_mul(out=sc[:, i:i + 1], in0=inv_s,
                                        scalar1=m)

        sqx = singles.tile([PAIRS, W], F32)
        sqy12 = singles.tile([PAIRS, W], F32)
        nc.scalar.activation(out=sqx, in_=iot,
                             func=mybir.ActivationFunctionType.Square,
                             bias=neg_m[:, 0:1], scale=1.0)
        nc.scalar.activation(out=sqy12, in_=iot,
                             func=mybir.ActivationFunctionType.Square,
                             bias=neg_m[:, 1:2], scale=1.0)
        mnx = singles.tile([PAIRS, 1], F32)
        mny = singles.tile([PAIRS, 1], F32)
        nc.vector.tensor_reduce(out=mnx, in_=sqx, axis=mybir.AxisListType.X,
                                op=mybir.AluOpType.min)
        nc.vector.tensor_reduce(out=mny, in_=sqy12, axis=mybir.AxisListType.X,
                                op=mybir.AluOpType.min)
        # fold BOTH min corrections into the x profile:
        #   exm[h] = exp(sc_x*sqx[h] + inv_s/2*mnx + inv_s/40*mny)
        bx = singles.tile([PAIRS, 1], F32)
        by = singles.tile([PAIRS, 1], F32)
        b2 = singles.tile([PAIRS, 1], F32)
        nc.vector.tensor_mul(out=bx, in0=mnx, in1=sc[:, 1:2])
        nc.vector.tensor_mul(out=by, in0=mny, in1=sc[:, 2:3])
        nc.vector.tensor_add(out=b2, in0=bx, in1=by)
        exm = singles.tile([PAIRS, W], F32)
        nc.scalar.activation(out=exm, in_=sqx,
                             func=mybir.ActivationFunctionType.Exp,
                             bias=b2, scale=sc[:, 0:1])

        # ---- PE-transpose x profile to per-partition scalars -------------
        ident = singles.tile([PAIRS, PAIRS], F32)
        make_identity(nc, ident)
        ext = singles.tile([P, CHUNKS, PAIRS], F32)
        for c in range(CHUNKS):
            pt = psum.tile([P, PAIRS], F32)
            nc.tensor.transpose(pt, exm[:, c * P:(c + 1) * P], ident)
            nc.vector.tensor_copy(out=ext[:, c, :], in_=pt)

        # ---- ingredients broadcast to all 128 partitions -----------------
        lab128 = singles.tile([P, BPC * 2 * NCLS], F32)
        lsrc = labels[:, :].rearrange("b t -> (b t)")
        nc.gpsimd.dma_start(
            out=lab128,
            in_=bass.AP(tensor=lsrc.tensor, offset=lsrc.offset,
                        ap=[[0, P], [1, BPC * 2 * NCLS]]),
        )
        lw128 = singles.tile([P, 1], F32)
        nc.gpsimd.dma_start(out=lw128, in_=logw[:, :].to_broadcast((P, 1)))

        # nmy128[:, p] = -my_p on every partition
        nmy128 = singles.tile([P, PAIRS], F32)
        nc.vector.tensor_scalar_mul(
            out=nmy128,
            in0=lab128[:, :].rearrange("p (q two) -> p q two", two=2)[:, :, 1],
            scalar1=-float(H))
        t128 = singles.tile([P, 1], F32)
        nc.vector.tensor_scalar(out=t128, in0=lw128, scalar1=-1.0,
                                scalar2=-LN_H, op0=mybir.AluOpType.mult,
                                op1=mybir.AluOpType.add)
        inv128 = singles.tile([P, 1], F32)
        nc.scalar.activation(out=inv128, in_=t128,
                             func=mybir.ActivationFunctionType.Exp,
                             bias=0.0, scale=1.0)
        scy128 = singles.tile([P, 1], F32)
        nc.vector.tensor_scalar_mul(out=scy128, in0=inv128, scalar1=-0.025)

        iog_i = singles.tile([P, W], mybir.dt.int32)
        nc.gpsimd.iota(iog_i, pattern=[[1, W]], base=0, channel_multiplier=0)
        iog = singles.tile([P, W], F32)
        nc.vector.tensor_copy(out=iog, in_=iog_i)

        # ---- main loop ---------------------------------------------------
        for p in range(PAIRS):
            sq = ybuf.tile([P, W], F32, tag="sq")
            nc.scalar.activation(out=sq, in_=iog,
                                 func=mybir.ActivationFunctionType.Square,
                                 bias=nmy128[:, p:p + 1], scale=1.0)
            eyb = ybuf.tile([P, W], F32, tag="eyb")
            nc.scalar.activation(out=eyb, in_=sq,
                                 func=mybir.ActivationFunctionType.Exp,
                                 bias=0.0, scale=scy128)
            st = stage.tile([P, CHUNKS, W], F32)
            for c in range(CHUNKS):
                nc.vector.tensor_scalar_mul(out=st[:, c, :], in0=eyb,
                                            scalar1=ext[:, c, p:p + 1])
            # rows of pair p are h = c*128 + par ; DRAM side iterates
            # (par, c, w) to match the SBUF tile layout.
            nc.sync.dma_start(
                out=out[p * H:(p + 1) * H, :].rearrange(
                    "(c par) w -> par c w", par=P),
                in_=st,
            )
    nc.finalize()
    return nc


LAST_RESULTS = None  # BassKernelResults of the most recent kernel() call


def kernel(x: np.ndarray, labels: np.ndarray,
           log_weight: np.ndarray, **run_kwargs) -> np.ndarray:
    global LAST_RESULTS
    del x  # only its (hardcoded) shape matters
    nc = build_bass()
    labels = np.ascontiguousarray(labels, dtype=np.float32)
    lw = np.ascontiguousarray(log_weight, dtype=np.float32).reshape(1, 1)
    in_maps = [
        {"labels": labels[i * BPC:(i + 1) * BPC], "log_weight": lw}
        for i in range(N_CORES)
    ]
    res = run_bass_kernel_spmd(nc, in_maps, core_ids=list(range(N_CORES)),
                               **run_kwargs)
    LAST_RESULTS = res
    outs = [r["out"].reshape(BPC, NCLS, H, W) for r in res.results]
    return np.concatenate(outs, axis=0)


if __name__ == "__main__":
    rng = np.random.default_rng(0)
    x = rng.standard_normal((B, CH, H, W), dtype=np.float32)
    labels = rng.random((B, 2 * NCLS), dtype=np.float32)
    lw = rng.random((1, 1, 1, 1), dtype=np.float32)
    y = kernel(x=x, labels=labels, log_weight=lw)
    print(y.shape, y.dtype, y.min(), y.max())
